# revision 1
# baseline (speedup 1.0000x reference)
"""AttnAdaIN Trainium2 kernel.

Computation (per batch b):
    F = f_w @ CK + f_b ; G = g_w @ SK + g_b ; Hh = h_w @ STY + h_b   (1x1 convs)
    S = softmax_k(F^T G)          [HW, HW]
    mean = S @ Hh^T ; second = S @ (Hh^T)^2
    std = sqrt(relu(second - mean^2))
    out = std * mvn(content) + mean      (mvn: per-channel mean/var norm, ddof=1)

Kernel strategy (8 NeuronCores, SPMD):
    core i -> (batch b = i//2, query-half h = i%2): 2048 query pixels x 4096 keys.
    Scores: S_pre = CK^T (W^T' SK) with W^T' = g_w^T f_w precomputed host-side,
    so no transposes are needed on-chip. Flash loop with score tiles in
    [k_part=128, q_free=256] orientation; PV matmuls use V-chunks as the
    stationary operand producing accumulators directly in [c, q] orientation
    (the output layout). Softmax runs without max-subtraction (scores are
    O(+-30): exp stays in fp32 range; any global shift cancels in the ratio).
    Denominator accumulated by a ones-vector matmul; 1/d and sqrt are computed
    on ScalarE with a single activation table set via exp/ln.
"""

import sys
import time

for _p in ("/opt/trn_rl_repo", "/opt/trn_rl_repo/concourse"):
    if _p not in sys.path:
        sys.path.insert(0, _p)

import contextlib

import numpy as np

import concourse.bacc as bacc
import concourse.mybir as mybir
import concourse.tile as tile
from concourse.bass_utils import run_bass_kernel_spmd

F32 = mybir.dt.float32
F32R = mybir.dt.float32r
AF = mybir.ActivationFunctionType
ALU = mybir.AluOpType


def build_program(C=512, HW=4096, Q=2048, q_tile=256, with_score_bias=False,
                  with_v_bias=False, n_cores=8):
    """Build + compile the per-core Bass program."""
    assert C % 128 == 0 and HW % 512 == 0 and Q % q_tile == 0
    CC = C // 128          # channel chunks
    NK = HW // 128         # key tiles (flash loop)
    NKS = HW // 512        # 512-wide key slices (G'' precompute)
    NQ = Q // q_tile       # query tiles
    NB = (CC + 1) // 2     # psum accumulator banks per moment (2 c-chunks/bank)
    assert (CC % 2 == 0 and 2 * q_tile <= 512) or CC == 1
    assert 2 * NB + 3 <= 8, "PSUM budget exceeded"

    nc = bacc.Bacc("TRN2", target_bir_lowering=False, debug=False,
                   num_devices=n_cores)

    ck = nc.dram_tensor("ck", [C, Q], F32R, kind="ExternalInput")
    sk = nc.dram_tensor("sk", [C, HW], F32R, kind="ExternalInput")
    sty = nc.dram_tensor("sty", [C, HW], F32R, kind="ExternalInput")
    # content, pre-rolled host-side so cols [0, Q) are this core's queries
    # while all HW pixels are present (channel statistics need all of them)
    ct = nc.dram_tensor("ct", [C, HW], F32, kind="ExternalInput")
    wT = nc.dram_tensor("wT", [C, C], F32R, kind="ExternalInput")
    hwT = nc.dram_tensor("hwT", [C, C], F32R, kind="ExternalInput")
    onesk_d = nc.dram_tensor("onesk", [128, 1], F32R, kind="ExternalInput")
    onesr_d = nc.dram_tensor("onesr", [1, 128], F32R, kind="ExternalInput")
    if with_score_bias:
        rbias = nc.dram_tensor("rbias", [1, HW], F32R, kind="ExternalInput")
    if with_v_bias:
        hb = nc.dram_tensor("hb", [1, C], F32R, kind="ExternalInput")
    out = nc.dram_tensor("out", [C, Q], F32, kind="ExternalOutput")

    ckr = ck.rearrange("(c p) q -> c p q", p=128)    # [CC, 128, Q]
    skr = sk.rearrange("(c p) k -> c p k", p=128)
    styr = sty.rearrange("(c p) k -> c p k", p=128)
    ctr = ct.rearrange("(c p) k -> c p k", p=128)
    wTr = wT.rearrange("(c p) a -> c p a", p=128)
    hwTr = hwT.rearrange("(c p) a -> c p a", p=128)
    outr = out.rearrange("(c p) q -> c p q", p=128)

    with tile.TileContext(nc) as tc, contextlib.ExitStack() as ctx:
        persist = ctx.enter_context(tc.tile_pool(name="persist", bufs=1))
        ckpool = ctx.enter_context(tc.tile_pool(name="ckpool", bufs=2))
        ppool = ctx.enter_context(tc.tile_pool(name="ppool", bufs=4))
        v2pool = ctx.enter_context(tc.tile_pool(name="v2pool", bufs=4))
        epool = ctx.enter_context(tc.tile_pool(name="epool", bufs=2))
        opool = ctx.enter_context(tc.tile_pool(name="opool", bufs=2))
        ps_st = ctx.enter_context(
            tc.tile_pool(name="ps_st", bufs=3, space="PSUM"))
        ps_acc = ctx.enter_context(
            tc.tile_pool(name="ps_acc", bufs=1, space="PSUM"))
        ps_d = ctx.enter_context(
            tc.tile_pool(name="ps_d", bufs=1, space="PSUM"))
        dpool = ctx.enter_context(
            tc.tile_pool(name="dpool", bufs=2, space="DRAM"))

        # ---- constants ----
        ones_k = persist.tile([128, 1], F32R, tag="ones_k")
        nc.sync.dma_start(out=ones_k, in_=onesk_d[:])
        ones_r = persist.tile([1, 128], F32R, tag="ones_r")
        nc.sync.dma_start(out=ones_r, in_=onesr_d[:])
        eps_sb = persist.tile([128, 1], F32, tag="eps")
        nc.vector.memset(eps_sb, 1e-5)
        shift_sb = persist.tile([128, 1], F32, tag="shift")
        nc.vector.memset(shift_sb, -30.0)

        g2 = persist.tile([128, CC, HW], F32R, tag="g2")
        vsb = persist.tile([128, NK, C], F32R, tag="v")
        mu = persist.tile([128, CC], F32, tag="mu")
        rstd = persist.tile([128, CC], F32, tag="rstd")
        if with_score_bias:
            r_sb = persist.tile([1, HW], F32R, tag="rbias")
            nc.sync.dma_start(out=r_sb, in_=rbias[:])
        if with_v_bias:
            hb_sb = persist.tile([1, C], F32R, tag="hb")
            nc.sync.dma_start(out=hb_sb, in_=hb[:])

        # ---- phase 0: weights, content stats, G'' and V precompute ----
        with tc.tile_pool(name="ph0", bufs=1) as ph0, \
             tc.tile_pool(name="ph0s", bufs=2) as ph0s:
            wT_sb = ph0.tile([128, CC, C], F32R, tag="wT")
            hwT_sb = ph0.tile([128, CC, C], F32R, tag="hwT")
            for c in range(CC):
                nc.sync.dma_start(out=wT_sb[:, c, :], in_=wTr[c])
                nc.sync.dma_start(out=hwT_sb[:, c, :], in_=hwTr[c])

            # G'' = W^T' SK  (score stationary operand), layout [c, k]
            for ks in range(2 * NKS):
                sl = slice(ks * 256, (ks + 1) * 256)
                sks = ph0s.tile([128, CC, 256], F32R, tag="sk_stream")
                for b in range(CC):
                    nc.sync.dma_start(out=sks[:, b, :], in_=skr[b][:, sl])
                for a in range(CC):
                    gps = ps_st.tile([128, 256], F32, tag="st", name="gps")
                    for b in range(CC):
                        nc.tensor.matmul(
                            gps,
                            lhsT=wT_sb[:, b, a * 128:(a + 1) * 128],
                            rhs=sks[:, b, :],
                            start=(b == 0), stop=(b == CC - 1))
                    nc.scalar.copy(out=g2[:, a, sl], in_=gps)

            # V = STY^T hwT  ([k, c] in 128-row blocks)
            for kt in range(NK):
                sl = slice(kt * 128, (kt + 1) * 128)
                sts = ph0s.tile([128, CC, 128], F32R, tag="sty_stream")
                for b in range(CC):
                    nc.sync.dma_start(out=sts[:, b, :], in_=styr[b][:, sl])
                vps = ps_st.tile([128, 512], F32, tag="st")
                for b in range(CC):
                    nc.tensor.matmul(vps[:, :C],
                                     lhsT=sts[:, b, :],
                                     rhs=hwT_sb[:, b, :],
                                     start=(b == 0), stop=(b == CC - 1))
                if with_v_bias:
                    nc.tensor.matmul(vps[:, :C],
                                     lhsT=ones_r,
                                     rhs=hb_sb,
                                     start=False, stop=True,
                                     skip_group_check=True)
                nc.scalar.copy(out=vsb[:, kt, :], in_=vps[:, :C])

            # content statistics (mean / rstd per channel over all HW pixels)
            BSF = nc.vector.BN_STATS_FMAX
            CH = min(HW, 512)          # stream chunk
            nsub = HW // BSF
            spc = CH // BSF             # stat subgroups per chunk
            for c in range(CC):
                stats = epool.tile([128, nsub, nc.vector.BN_STATS_DIM], F32,
                                   tag="bn_stats", bufs=1)
                for i in range(HW // CH):
                    ctile = ph0s.tile([128, CH], F32, tag="ct_stream")
                    nc.sync.dma_start(out=ctile,
                                      in_=ctr[c][:, i * CH:(i + 1) * CH])
                    for s in range(spc):
                        nc.vector.bn_stats(
                            out=stats[:, i * spc + s, :],
                            in_=ctile[:, s * BSF:(s + 1) * BSF])
                mv = epool.tile([128, nc.vector.BN_AGGR_DIM], F32, tag="bn_mv", bufs=1)
                nc.vector.bn_aggr(out=mv, in_=stats)
                nc.vector.tensor_copy(out=mu[:, c:c + 1], in_=mv[:, 0:1])
                # rstd = (var * HW/(HW-1) + eps) ** -0.5 via exp(-0.5*ln(x))
                lnv = epool.tile([128, 1], F32, tag="lnv1", bufs=1)
                nc.scalar.activation(out=lnv, in_=mv[:, 1:2], func=AF.Ln,
                                     scale=float(HW) / (HW - 1), bias=eps_sb)
                nc.scalar.activation(out=rstd[:, c:c + 1], in_=lnv,
                                     func=AF.Exp, scale=-0.5)


        # ---- flash main loop ----
        for qt in range(NQ):
            qsl = slice(qt * q_tile, (qt + 1) * q_tile)
            ckq = ckpool.tile([128, CC, q_tile], F32R, tag="ckq")
            for c in range(CC):
                nc.sync.dma_start(out=ckq[:, c, :], in_=ckr[c][:, qsl])

            acc1 = [ps_acc.tile([128, 512], F32, tag=f"acc1_{i}",
                                name=f"acc1_{i}") for i in range(NB)]
            acc2 = [ps_acc.tile([128, 512], F32, tag=f"acc2_{i}",
                                name=f"acc2_{i}") for i in range(NB)]
            dps = ps_d.tile([1, q_tile], F32, tag="d")

            def acc_ap(accs, c):
                return accs[c // 2][:, (c % 2) * q_tile:(c % 2 + 1) * q_tile]

            # NOTE: start=True clears has_written bits for the WHOLE psum
            # bank, so each bank (2 c-chunks) forms a single accumulation
            # group: only its first matmul sets start.
            def emit_pv(kt, p, v2):
                nc.tensor.matmul(dps, lhsT=ones_k, rhs=p,
                                 start=(kt == 0), stop=(kt == NK - 1),
                                 skip_group_check=True)
                for acc, lhs in ((acc1, vsb[:, kt, :]), (acc2, v2)):
                    for c in range(CC):
                        csl = slice(c * 128, (c + 1) * 128)
                        nc.tensor.matmul(acc_ap(acc, c),
                                         lhsT=lhs[:, csl],
                                         rhs=p,
                                         start=(kt == 0 and c % 2 == 0),
                                         stop=(kt == NK - 1 and
                                               (c % 2 == 1 or c == CC - 1)),
                                         skip_group_check=True)

            # software pipeline: QK(kt) is emitted before PV(kt-1) so the PE
            # has score matmuls to run while ScalarE computes exp(kt-1).
            pending = []
            for kt in range(NK):
                ksl = slice(kt * 128, (kt + 1) * 128)
                st = ps_st.tile([128, q_tile], F32, tag="st")
                for c in range(CC):
                    nc.tensor.matmul(st,
                                     lhsT=g2[:, c, ksl],
                                     rhs=ckq[:, c, :],
                                     start=(c == 0),
                                     stop=(c == CC - 1 and not with_score_bias))
                if with_score_bias:
                    nc.tensor.matmul(st, lhsT=r_sb[:, ksl],
                                     rhs=ones_r[:, :q_tile],
                                     start=False, stop=True,
                                     skip_group_check=True)
                p = ppool.tile([128, q_tile], F32R, tag="p")
                nc.scalar.activation(out=p, in_=st, func=AF.Exp, bias=shift_sb)
                v2 = v2pool.tile([128, C], F32R, tag="v2")
                nc.gpsimd.tensor_mul(v2, vsb[:, kt, :], vsb[:, kt, :])
                pending.append((kt, p, v2))
                if len(pending) > 2:
                    emit_pv(*pending.pop(0))
            for item in pending:
                emit_pv(*item)

            # ---- epilogue for this q_tile ----
            rd = epool.tile([1, q_tile], F32, tag="rd", bufs=1)
            nc.vector.reciprocal(out=rd, in_=dps)
            rd_dram = dpool.tile([1, q_tile], F32, tag="rd_dram")
            nc.sync.dma_start(out=rd_dram, in_=rd)
            rdb = epool.tile([128, q_tile], F32, tag="rdb", bufs=1)
            nc.sync.dma_start(out=rdb,
                              in_=rd_dram.to_broadcast([128, q_tile]))

            avs, a2s = [], []
            for c in range(CC):
                av = epool.tile([128, q_tile], F32, tag=f"av{c}", name=f"av{c}", bufs=1)
                nc.scalar.copy(out=av, in_=acc_ap(acc1, c))
                a2 = epool.tile([128, q_tile], F32, tag=f"a2{c}", name=f"a2{c}", bufs=1)
                nc.scalar.copy(out=a2, in_=acc_ap(acc2, c))
                avs.append(av)
                a2s.append(a2)

            for c in range(CC):
                ctq = epool.tile([128, q_tile], F32, tag="ctq")
                nc.sync.dma_start(out=ctq, in_=ctr[c][:, qsl])
                mean = avs[c]
                nc.vector.tensor_mul(mean, avs[c], rdb)
                e2 = a2s[c]
                nc.vector.tensor_mul(e2, a2s[c], rdb)
                var = epool.tile([128, q_tile], F32, tag="var", bufs=1)
                nc.vector.tensor_mul(var, mean, mean)
                nc.vector.scalar_tensor_tensor(
                    out=var, in0=var, scalar=-1.0, in1=e2,
                    op0=ALU.mult, op1=ALU.add)
                nc.vector.tensor_scalar_max(var, var, 1e-38)
                std = var
                nc.scalar.activation(out=std, in_=var, func=AF.Ln)
                nc.scalar.activation(out=std, in_=std, func=AF.Exp, scale=0.5)
                normc = epool.tile([128, q_tile], F32, tag="normc", bufs=1)
                nc.vector.tensor_scalar(
                    out=normc, in0=ctq,
                    scalar1=mu[:, c:c + 1], scalar2=rstd[:, c:c + 1],
                    op0=ALU.subtract, op1=ALU.mult)
                o = opool.tile([128, q_tile], F32, tag="o")
                nc.vector.tensor_mul(o, std, normc)
                nc.vector.tensor_add(o, o, mean)
                nc.sync.dma_start(out=outr[c][:, qsl], in_=o)

    # Force exp/ln/copy onto the shared natural_log_exp_and_others table
    # set: the default per-function choice alternates exp_and_others <->
    # natural_log, costing ~2.7us per ACT_TABLE_LOAD, dozens of times.
    import concourse.bacc as bacc_mod
    _orig_tables = bacc_mod.get_activation_tables
    _keep = "natural_log_exp_and_others"
    _strip = {AF.Exp, AF.Ln, AF.Copy, AF.Identity}

    def _patched_tables(arch):
        t = _orig_tables(arch)
        for name, fns in t.items():
            if name != _keep:
                t[name] = fns - _strip
        return t

    bacc_mod.get_activation_tables = _patched_tables
    try:
        nc.compile()
    finally:
        bacc_mod.get_activation_tables = _orig_tables
    return nc


_PROGRAM_CACHE = {}


def _get_program(key):
    if key not in _PROGRAM_CACHE:
        with_r, with_hb = key
        _PROGRAM_CACHE[key] = build_program(
            with_score_bias=with_r, with_v_bias=with_hb)
    return _PROGRAM_CACHE[key]


def make_in_maps(content, style, content_key, style_key, f_w, f_b, g_w, g_b,
                 h_w, h_b):
    content = np.asarray(content, np.float32)
    style = np.asarray(style, np.float32)
    content_key = np.asarray(content_key, np.float32)
    style_key = np.asarray(style_key, np.float32)

    def rnd22(x):
        # round-to-nearest fp22 (e10m11): the PE truncates fp32r operands to
        # 11 mantissa bits; pre-rounding host-side makes that truncation exact
        # and unbiased.
        xi = np.ascontiguousarray(x, np.float32).view(np.uint32)
        return ((xi + np.uint32(0x800)) & np.uint32(0xFFFFF000)).view(np.float32)

    style = rnd22(style)
    content_key = rnd22(content_key)
    style_key = rnd22(style_key)
    B, C, H, W = content.shape
    HW = H * W
    Q = HW // 2
    f64 = np.float64
    wT = rnd22((np.asarray(g_w, f64).T @ np.asarray(f_w, f64))
               .astype(np.float32))
    hwT = rnd22(np.ascontiguousarray(np.asarray(h_w, np.float32).T))
    with_r = bool(np.any(f_b))
    with_hb = bool(np.any(h_b))
    u = np.asarray(g_w, f64).T @ np.asarray(f_b, f64)      # [C]
    in_maps = []
    for core in range(2 * B):
        b, h = divmod(core, 2)
        sk_b = np.ascontiguousarray(style_key[b].reshape(C, HW))
        sty_b = np.ascontiguousarray(style[b].reshape(C, HW))
        ct_b = content[b].reshape(C, HW)
        ct_roll = np.ascontiguousarray(np.roll(ct_b, -h * Q, axis=1))
        ck_b = np.ascontiguousarray(
            content_key[b].reshape(C, HW)[:, h * Q:(h + 1) * Q])
        m = {"ck": ck_b, "sk": sk_b, "sty": sty_b, "ct": ct_roll,
             "wT": wT, "hwT": hwT,
             "onesk": np.ones((128, 1), np.float32),
             "onesr": np.ones((1, 128), np.float32)}
        if with_r:
            r = (u @ sk_b.astype(f64)).astype(np.float32)[None, :]
            m["rbias"] = np.ascontiguousarray(r)
        if with_hb:
            m["hb"] = np.ascontiguousarray(
                np.asarray(h_b, np.float32)[None, :])
        in_maps.append(m)
    return in_maps, (with_r, with_hb)


def assemble(results, B=4, C=512, H=64, W=64):
    HW = H * W
    Q = HW // 2
    out = np.empty((B, C, HW), np.float32)
    for core in range(2 * B):
        b, h = divmod(core, 2)
        out[b][:, h * Q:(h + 1) * Q] = results[core]["out"]
    return out.reshape(B, C, H, W)


def kernel(**inputs):
    in_maps, key = make_in_maps(**inputs)
    nc = _get_program(key)
    res = run_bass_kernel_spmd(nc, in_maps, list(range(8)))
    return assemble(res.results)


if __name__ == "__main__":
    rng = np.random.default_rng(0)
    B, C, H, W = 4, 512, 64, 64
    inputs = {
        "content": rng.standard_normal((B, C, H, W)).astype(np.float32),
        "style": rng.standard_normal((B, C, H, W)).astype(np.float32),
        "content_key": rng.standard_normal((B, C, H, W)).astype(np.float32),
        "style_key": rng.standard_normal((B, C, H, W)).astype(np.float32),
        "f_w": (rng.standard_normal((C, C)) * 0.02).astype(np.float32),
        "f_b": np.zeros(C, np.float32),
        "g_w": (rng.standard_normal((C, C)) * 0.02).astype(np.float32),
        "g_b": np.zeros(C, np.float32),
        "h_w": (rng.standard_normal((C, C)) * 0.02).astype(np.float32),
        "h_b": np.zeros(C, np.float32),
    }
    t0 = time.time()
    out = kernel(**inputs)
    print("kernel done", out.shape, out.dtype, time.time() - t0)

